# revision 26
# baseline (speedup 1.0000x reference)
"""DWTFM fused kernel for Trainium2 (Bass/Tile), 8-core data parallel.

Math: the reference computes LL of dwt(x0), LH/HL/HH of dwt(x1), then idwt.
Algebraically this collapses to a local 2x2 stencil:

    out[2i+r, 2j+s] = x1[2i+r, 2j+s] + 0.25 * sum_{r',s'} (x0 - x1)[2i+r', 2j+s']

i.e. out = x1 + upsample2x2(blockmean2x2(x0 - x1)), independently per (b, c).

Layout: per core, the [2, 3, 512, 512] shard is viewed as [1536, 1024] where
each row is one "block-row" = two consecutive image rows concatenated
([even_row(512) | odd_row(512)]). 2x2 blocks never straddle rows of this view.
"""

import numpy as np

_B, _C, _H, _W = 16, 3, 512, 512
_NCORES = 8
_BPC = _B // _NCORES          # batch entries per core
_ROWS = _BPC * _C * _H // 2   # 1536 block-rows per core
_COLS = 2 * _W                # 1024
_P = 128                      # partitions per tile
_NT = _ROWS // _P             # 12 chunks per core


def _build(
    reps: int = 1,
    loop_iters: int | None = None,
    rpp: int = 1,
    bufs: int = 3,
    store_engine: str = "sync",
    load_engines: tuple = ("sync", "sync"),
    alt_store: bool = False,
    fuse_final: bool = False,
):
    """Emit the Bass program.

    rpp: block-rows per partition (1 -> 512 KB DMAs, 2 -> 1 MB, ...).
    reps>1 unrolls the full sweep back-to-back; loop_iters wraps that in a
    hardware For_i loop (same DRAM I/O every iteration) - both used only
    for slope-based HW timing.
    """
    import contextlib

    import concourse.bacc as bacc
    import concourse.mybir as mybir
    from concourse.tile import TileContext

    f32 = mybir.dt.float32
    W = _W
    J = W // 2  # 256 blocks per image row
    C = rpp * _COLS          # free size of one i/o tile
    n_chunks = _ROWS // (_P * rpp)

    nc = bacc.Bacc("TRN2", target_bir_lowering=False)
    # Row r of the [_ROWS/rpp, C] view packs rpp consecutive block-rows.
    x0 = nc.dram_tensor("x0", [_ROWS // rpp, C], f32, kind="ExternalInput").ap()
    x1 = nc.dram_tensor("x1", [_ROWS // rpp, C], f32, kind="ExternalInput").ap()
    y = nc.dram_tensor("y", [_ROWS // rpp, C], f32, kind="ExternalOutput").ap()

    with TileContext(nc) as tc:
        with tc.tile_pool(name="pool", bufs=bufs) as pool:
            store_eng = getattr(nc, store_engine)
            load0 = getattr(nc, load_engines[0])
            load1 = getattr(nc, load_engines[1])

            def emit_chunk(k):
                r = k * _P
                t0 = pool.tile([_P, C], f32, name="t0")
                t1 = pool.tile([_P, C], f32, name="t1")
                load0.dma_start(out=t0[:], in_=x0[r : r + _P, :])
                load1.dma_start(out=t1[:], in_=x1[r : r + _P, :])

                # Per-partition layout: [i:rpp, r2:2, w:W].
                t04 = t0.rearrange("p (i r2 w) -> p i r2 w", r2=2, w=W)
                t14 = t1.rearrange("p (i r2 w) -> p i r2 w", r2=2, w=W)

                # Vertical pair sums per input (each DVE op waits on only
                # one DMA), then subtract:
                # v = (x0_even + x0_odd) - (x1_even + x1_odd).
                a = pool.tile([_P, rpp * W], f32, name="a")
                a3 = a.rearrange("p (i w) -> p i w", w=W)
                nc.vector.tensor_add(out=a3[:], in0=t04[:, :, 0], in1=t04[:, :, 1])
                b = pool.tile([_P, rpp * W], f32, name="b")
                b3 = b.rearrange("p (i w) -> p i w", w=W)
                nc.vector.tensor_add(out=b3[:], in0=t14[:, :, 0], in1=t14[:, :, 1])
                v = pool.tile([_P, rpp * W], f32, name="v")
                nc.vector.tensor_sub(out=v[:], in0=a[:], in1=b[:])
                # m[p, i, j] = v[p, i, 2j] + v[p, i, 2j+1] (horizontal sum)
                m = pool.tile([_P, rpp * J], f32, name="m")
                m3 = m.rearrange("p (i j) -> p i j", j=J)
                v4 = v.rearrange("p (i j s) -> p i j s", j=J, s=2)
                nc.vector.tensor_add(out=m3[:], in0=v4[:, :, :, 0], in1=v4[:, :, :, 1])

                yt = pool.tile([_P, C], f32, name="yt")
                if fuse_final:
                    # y = (m_bcast * 0.25) + x1 as 3D ScalarTensorTensor ops
                    # on DVE (one per (block-row, row-of-pair)); no ACT
                    # compute at all, so the ACT ring only issues stores.
                    y5 = yt.rearrange(
                        "p (i r2 j s) -> p i r2 j s", r2=2, j=J, s=2
                    )
                    x5 = t1.rearrange(
                        "p (i r2 j s) -> p i r2 j s", r2=2, j=J, s=2
                    )
                    for i in range(rpp):
                        mb = m3[:, i].unsqueeze(2).broadcast_to([_P, J, 2])
                        for r2 in range(2):
                            nc.vector.scalar_tensor_tensor(
                                y5[:, i, r2],
                                mb,
                                0.25,
                                x5[:, i, r2],
                                mybir.AluOpType.mult,
                                mybir.AluOpType.add,
                            )
                else:
                    # mu[p, i, 2j+s] = 0.25 * m[p, i, j] (upsample + scale)
                    # on the Scalar engine; one op per block-row i (ACT APs
                    # max 3D).
                    mu = pool.tile([_P, rpp * W], f32, name="mu")
                    mu4 = mu.rearrange("p (i j s) -> p i j s", j=J, s=2)
                    for i in range(rpp):
                        mb = m3[:, i].unsqueeze(2).broadcast_to([_P, J, 2])
                        nc.scalar.activation(
                            mu4[:, i],
                            mb,
                            mybir.ActivationFunctionType.Copy,
                            scale=0.25,
                        )

                    # y = x1 + mu broadcast over the row-of-pair axis; one
                    # 3D TensorTensor per block-row i.
                    y4 = yt.rearrange("p (i r2 w) -> p i r2 w", r2=2, w=W)
                    mu3 = mu.rearrange("p (i w) -> p i w", w=W)
                    for i in range(rpp):
                        mub = mu3[:, i].unsqueeze(1).broadcast_to([_P, 2, W])
                        nc.vector.tensor_add(
                            out=y4[:, i], in0=t14[:, i], in1=mub
                        )
                se = (
                    getattr(nc, ("sync", "scalar")[k % 2]) if alt_store else store_eng
                )
                se.dma_start(out=y[r : r + _P, :], in_=yt[:])

            loop_cm = (
                tc.For_i(0, loop_iters, 1)
                if loop_iters is not None
                else contextlib.nullcontext()
            )
            with loop_cm:
                for _rep in range(reps):
                    for k in range(n_chunks):
                        emit_chunk(k)
    nc.compile()
    return nc


def _make_runner(nc):
    """Jitted 8-core shard_map callable wrapping the Bass NEFF. Mirrors
    concourse.bass2jax.run_bass_via_pjrt but reusable across calls (no
    output-buffer donation, cached jit)."""
    import jax
    import concourse.mybir as mybir
    from concourse import bass2jax
    from jax.experimental.shard_map import shard_map
    from jax.sharding import Mesh, PartitionSpec

    bass2jax.install_neuronx_cc_hook()

    partition_name = (
        nc.partition_id_tensor.name if nc.partition_id_tensor else None
    )
    in_names = []
    out_names = []
    out_avals = []
    for alloc in nc.m.functions[0].allocations:
        if not isinstance(alloc, mybir.MemoryLocationSet):
            continue
        name = alloc.memorylocations[0].name
        if alloc.kind == "ExternalInput":
            if name != partition_name:
                in_names.append(name)
        elif alloc.kind == "ExternalOutput":
            out_names.append(name)
            out_avals.append(
                jax.core.ShapedArray(
                    tuple(alloc.tensor_shape), mybir.dt.np(alloc.dtype)
                )
            )
    assert in_names == ["x0", "x1"] and out_names == ["y"], (in_names, out_names)
    all_in_names = tuple(in_names + out_names)
    if partition_name is not None:
        all_in_names = all_in_names + (partition_name,)

    def _body(*args):
        operands = list(args)
        if partition_name is not None:
            operands.append(bass2jax.partition_id_tensor())
        outs = bass2jax._bass_exec_p.bind(
            *operands,
            out_avals=tuple(out_avals),
            in_names=all_in_names,
            out_names=tuple(out_names),
            lowering_input_output_aliases=(),
            sim_require_finite=True,
            sim_require_nnan=True,
            nc=nc,
        )
        return tuple(outs)

    devices = jax.devices()[:_NCORES]
    mesh = Mesh(np.asarray(devices), ("core",))
    n_args = len(in_names) + len(out_names)
    fn = jax.jit(
        shard_map(
            _body,
            mesh=mesh,
            in_specs=(PartitionSpec("core"),) * n_args,
            out_specs=(PartitionSpec("core"),) * len(out_names),
            check_rep=False,
        ),
        keep_unused=True,
    )
    return fn, mesh


_runners = {}

# Default config for the graded kernel() path: 1 MB DMAs (2 block-rows per
# partition), triple buffering, loads on the SP HWDGE ring, stores on the
# ACT ring (which does no compute - final adds are fused ScalarTensorTensor
# ops on DVE). Measured ~57-59 us per sweep across 8 cores (~320-330
# GB/s/core, ~90% of practical HBM-DMA peak).
_KERNEL_CFG = dict(rpp=2, bufs=3, store_engine="scalar", fuse_final=True)


def get_runner(reps: int = 1, loop_iters: int | None = None, **build_kw):
    """(fn, zeros, mesh, gshape) for the repeated sweep. reps=1 /
    loop_iters=None is the real kernel; other values exist for slope-based
    HW timing."""
    global _runners
    kw = dict(_KERNEL_CFG)
    kw.update(build_kw)
    key = (reps, loop_iters, tuple(sorted(kw.items())))
    if key not in _runners:
        import jax
        from jax.sharding import NamedSharding, PartitionSpec

        rpp = kw["rpp"]
        gshape = (_NCORES * _ROWS // rpp, rpp * _COLS)
        fn, mesh = _make_runner(_build(reps, loop_iters, **kw))
        zeros = jax.device_put(
            np.zeros(gshape, np.float32),
            NamedSharding(mesh, PartitionSpec("core")),
        )
        _runners[key] = (fn, zeros, mesh, gshape)
    return _runners[key]


def kernel(x0: np.ndarray, x1: np.ndarray) -> np.ndarray:
    fn, zeros, _mesh, gshape = get_runner(1)
    # Per-core shard c is x[c*_BPC:(c+1)*_BPC] reshaped; stacking the 8
    # shards along axis 0 is exactly the full tensor reshaped.
    g0 = np.ascontiguousarray(x0, dtype=np.float32).reshape(gshape)
    g1 = np.ascontiguousarray(x1, dtype=np.float32).reshape(gshape)
    (y,) = fn(g0, g1, zeros)
    return np.asarray(y).reshape(_B, _C, _H, _W)


# revision 31
# speedup vs baseline: 1.0705x; 1.0705x over previous
"""DWTFM fused kernel for Trainium2 (Bass/Tile), 8-core data parallel.

Math: the reference computes LL of dwt(x0), LH/HL/HH of dwt(x1), then idwt.
Algebraically this collapses to a local 2x2 stencil:

    out[2i+r, 2j+s] = x1[2i+r, 2j+s] + 0.25 * sum_{r',s'} (x0 - x1)[2i+r', 2j+s']

i.e. out = x1 + upsample2x2(blockmean2x2(x0 - x1)), independently per (b, c).

Layout: per core, the [2, 3, 512, 512] shard is viewed as [1536, 1024] where
each row is one "block-row" = two consecutive image rows concatenated
([even_row(512) | odd_row(512)]). 2x2 blocks never straddle rows of this view.
"""

import numpy as np

_B, _C, _H, _W = 16, 3, 512, 512
_NCORES = 8
_BPC = _B // _NCORES          # batch entries per core
_ROWS = _BPC * _C * _H // 2   # 1536 block-rows per core
_COLS = 2 * _W                # 1024
_P = 128                      # partitions per tile
_NT = _ROWS // _P             # 12 chunks per core


def _build(
    reps: int = 1,
    loop_iters: int | None = None,
    rpp: int = 1,
    bufs: int = 3,
    store_engine: str = "sync",
    load_engines: tuple = ("sync", "sync"),
    alt_store: bool = False,
    fuse_final: bool = False,
    load_bufs: int | None = None,
    yt_bufs: int | None = None,
    mode: str = "normal",
):
    """Emit the Bass program.

    rpp: block-rows per partition (1 -> 512 KB DMAs, 2 -> 1 MB, ...).
    reps>1 unrolls the full sweep back-to-back; loop_iters wraps that in a
    hardware For_i loop (same DRAM I/O every iteration) - both used only
    for slope-based HW timing.
    """
    import contextlib

    import concourse.bacc as bacc
    import concourse.mybir as mybir
    from concourse.tile import TileContext

    f32 = mybir.dt.float32
    W = _W
    J = W // 2  # 256 blocks per image row
    C = rpp * _COLS          # free size of one i/o tile
    n_chunks = _ROWS // (_P * rpp)

    nc = bacc.Bacc("TRN2", target_bir_lowering=False)
    # Row r of the [_ROWS/rpp, C] view packs rpp consecutive block-rows.
    x0 = nc.dram_tensor("x0", [_ROWS // rpp, C], f32, kind="ExternalInput").ap()
    x1 = nc.dram_tensor("x1", [_ROWS // rpp, C], f32, kind="ExternalInput").ap()
    y = nc.dram_tensor("y", [_ROWS // rpp, C], f32, kind="ExternalOutput").ap()

    with TileContext(nc) as tc:
        with tc.tile_pool(name="pool", bufs=bufs) as pool:
            store_eng = getattr(nc, store_engine)
            load0 = getattr(nc, load_engines[0])
            load1 = getattr(nc, load_engines[1])

            def emit_chunk(k):
                r = k * _P
                t0 = pool.tile([_P, C], f32, name="t0", bufs=load_bufs)
                t1 = pool.tile([_P, C], f32, name="t1", bufs=load_bufs)
                load0.dma_start(out=t0[:], in_=x0[r : r + _P, :])
                if mode == "copy":
                    # timing probe: 1 read + 1 write, no compute
                    store_eng.dma_start(out=y[r : r + _P, :], in_=t0[:])
                    return
                load1.dma_start(out=t1[:], in_=x1[r : r + _P, :])
                if mode == "loadonly":
                    # timing probe: reads only
                    return

                # Per-partition layout: [i:rpp, r2:2, w:W].
                t04 = t0.rearrange("p (i r2 w) -> p i r2 w", r2=2, w=W)
                t14 = t1.rearrange("p (i r2 w) -> p i r2 w", r2=2, w=W)

                # Vertical pair sums per input (each DVE op waits on only
                # one DMA), then subtract:
                # v = (x0_even + x0_odd) - (x1_even + x1_odd).
                a = pool.tile([_P, rpp * W], f32, name="a")
                a3 = a.rearrange("p (i w) -> p i w", w=W)
                nc.vector.tensor_add(out=a3[:], in0=t04[:, :, 0], in1=t04[:, :, 1])
                b = pool.tile([_P, rpp * W], f32, name="b")
                b3 = b.rearrange("p (i w) -> p i w", w=W)
                nc.vector.tensor_add(out=b3[:], in0=t14[:, :, 0], in1=t14[:, :, 1])
                v = pool.tile([_P, rpp * W], f32, name="v")
                nc.vector.tensor_sub(out=v[:], in0=a[:], in1=b[:])
                # m[p, i, j] = v[p, i, 2j] + v[p, i, 2j+1] (horizontal sum)
                m = pool.tile([_P, rpp * J], f32, name="m")
                m3 = m.rearrange("p (i j) -> p i j", j=J)
                v4 = v.rearrange("p (i j s) -> p i j s", j=J, s=2)
                nc.vector.tensor_add(out=m3[:], in0=v4[:, :, :, 0], in1=v4[:, :, :, 1])

                yt = pool.tile([_P, C], f32, name="yt", bufs=yt_bufs)
                if fuse_final:
                    # y = (m_bcast * 0.25) + x1 as 3D ScalarTensorTensor ops
                    # on DVE (one per (block-row, row-of-pair)); no ACT
                    # compute at all, so the ACT ring only issues stores.
                    y5 = yt.rearrange(
                        "p (i r2 j s) -> p i r2 j s", r2=2, j=J, s=2
                    )
                    x5 = t1.rearrange(
                        "p (i r2 j s) -> p i r2 j s", r2=2, j=J, s=2
                    )
                    for i in range(rpp):
                        mb = m3[:, i].unsqueeze(2).broadcast_to([_P, J, 2])
                        for r2 in range(2):
                            nc.vector.scalar_tensor_tensor(
                                y5[:, i, r2],
                                mb,
                                0.25,
                                x5[:, i, r2],
                                mybir.AluOpType.mult,
                                mybir.AluOpType.add,
                            )
                else:
                    # mu[p, i, 2j+s] = 0.25 * m[p, i, j] (upsample + scale)
                    # on the Scalar engine; one op per block-row i (ACT APs
                    # max 3D).
                    mu = pool.tile([_P, rpp * W], f32, name="mu")
                    mu4 = mu.rearrange("p (i j s) -> p i j s", j=J, s=2)
                    for i in range(rpp):
                        mb = m3[:, i].unsqueeze(2).broadcast_to([_P, J, 2])
                        nc.scalar.activation(
                            mu4[:, i],
                            mb,
                            mybir.ActivationFunctionType.Copy,
                            scale=0.25,
                        )

                    # y = x1 + mu broadcast over the row-of-pair axis; one
                    # 3D TensorTensor per block-row i.
                    y4 = yt.rearrange("p (i r2 w) -> p i r2 w", r2=2, w=W)
                    mu3 = mu.rearrange("p (i w) -> p i w", w=W)
                    for i in range(rpp):
                        mub = mu3[:, i].unsqueeze(1).broadcast_to([_P, 2, W])
                        nc.vector.tensor_add(
                            out=y4[:, i], in0=t14[:, i], in1=mub
                        )
                se = (
                    getattr(nc, ("sync", "scalar")[k % 2]) if alt_store else store_eng
                )
                se.dma_start(out=y[r : r + _P, :], in_=yt[:])

            loop_cm = (
                tc.For_i(0, loop_iters, 1)
                if loop_iters is not None
                else contextlib.nullcontext()
            )
            with loop_cm:
                for _rep in range(reps):
                    for k in range(n_chunks):
                        emit_chunk(k)
    nc.compile()
    return nc


def _make_runner(nc):
    """Jitted 8-core shard_map callable wrapping the Bass NEFF. Mirrors
    concourse.bass2jax.run_bass_via_pjrt but reusable across calls (no
    output-buffer donation, cached jit)."""
    import jax
    import concourse.mybir as mybir
    from concourse import bass2jax
    from jax.experimental.shard_map import shard_map
    from jax.sharding import Mesh, PartitionSpec

    bass2jax.install_neuronx_cc_hook()

    partition_name = (
        nc.partition_id_tensor.name if nc.partition_id_tensor else None
    )
    in_names = []
    out_names = []
    out_avals = []
    for alloc in nc.m.functions[0].allocations:
        if not isinstance(alloc, mybir.MemoryLocationSet):
            continue
        name = alloc.memorylocations[0].name
        if alloc.kind == "ExternalInput":
            if name != partition_name:
                in_names.append(name)
        elif alloc.kind == "ExternalOutput":
            out_names.append(name)
            out_avals.append(
                jax.core.ShapedArray(
                    tuple(alloc.tensor_shape), mybir.dt.np(alloc.dtype)
                )
            )
    assert in_names == ["x0", "x1"] and out_names == ["y"], (in_names, out_names)
    all_in_names = tuple(in_names + out_names)
    if partition_name is not None:
        all_in_names = all_in_names + (partition_name,)

    def _body(*args):
        operands = list(args)
        if partition_name is not None:
            operands.append(bass2jax.partition_id_tensor())
        outs = bass2jax._bass_exec_p.bind(
            *operands,
            out_avals=tuple(out_avals),
            in_names=all_in_names,
            out_names=tuple(out_names),
            lowering_input_output_aliases=(),
            sim_require_finite=True,
            sim_require_nnan=True,
            nc=nc,
        )
        return tuple(outs)

    devices = jax.devices()[:_NCORES]
    mesh = Mesh(np.asarray(devices), ("core",))
    n_args = len(in_names) + len(out_names)
    fn = jax.jit(
        shard_map(
            _body,
            mesh=mesh,
            in_specs=(PartitionSpec("core"),) * n_args,
            out_specs=(PartitionSpec("core"),) * len(out_names),
            check_rep=False,
        ),
        keep_unused=True,
    )
    return fn, mesh


_runners = {}

# Default config for the graded kernel() path: 1 MB DMAs (2 block-rows per
# partition), triple buffering, loads on the SP HWDGE ring, stores on the
# ACT ring (which does no compute - final adds are fused ScalarTensorTensor
# ops on DVE). Measured ~57-59 us per sweep across 8 cores (~320-330
# GB/s/core, ~90% of practical HBM-DMA peak).
_KERNEL_CFG = dict(rpp=2, bufs=3, store_engine="scalar", fuse_final=True)


def get_runner(reps: int = 1, loop_iters: int | None = None, **build_kw):
    """(fn, zeros, mesh, gshape) for the repeated sweep. reps=1 /
    loop_iters=None is the real kernel; other values exist for slope-based
    HW timing."""
    global _runners
    kw = dict(_KERNEL_CFG)
    kw.update(build_kw)
    key = (reps, loop_iters, tuple(sorted(kw.items())))
    if key not in _runners:
        import jax
        from jax.sharding import NamedSharding, PartitionSpec

        rpp = kw["rpp"]
        gshape = (_NCORES * _ROWS // rpp, rpp * _COLS)
        fn, mesh = _make_runner(_build(reps, loop_iters, **kw))
        zeros = jax.device_put(
            np.zeros(gshape, np.float32),
            NamedSharding(mesh, PartitionSpec("core")),
        )
        _runners[key] = (fn, zeros, mesh, gshape)
    return _runners[key]


def kernel(x0: np.ndarray, x1: np.ndarray) -> np.ndarray:
    fn, zeros, _mesh, gshape = get_runner(1)
    # Per-core shard c is x[c*_BPC:(c+1)*_BPC] reshaped; stacking the 8
    # shards along axis 0 is exactly the full tensor reshaped.
    g0 = np.ascontiguousarray(x0, dtype=np.float32).reshape(gshape)
    g1 = np.ascontiguousarray(x1, dtype=np.float32).reshape(gshape)
    (y,) = fn(g0, g1, zeros)
    return np.asarray(y).reshape(_B, _C, _H, _W)
